# revision 22
# baseline (speedup 1.0000x reference)
"""Normalized LSTM cell on 8 Trainium2 NeuronCores (Bass/Tile).

reference:
    wh = h0 @ w_hh + bias_h            # [B, 4H]
    wi = x  @ w_ih + bias_x            # [B, 4H]
    s  = LN(wh)*g_hh + b_hh + LN(wi)*g_ih + b_ih
    f, i, o, g = split(s, 4)
    c1 = sigmoid(f)*c0 + sigmoid(i)*tanh(g)
    h1 = sigmoid(o)*tanh(LN(c1)*g_c + b_c)
    return h1, c1

Sharding: data-parallel over batch, B=16384 -> 2048 rows/core.

Per-core kernel math (fast path: all gammas==1, betas==0, biases==0):
    LN(wh)+LN(wi) = r1*wh + r2*wi + c,   r=1/std, c=-(m1*r1+m2*r2)
                  = (wh + lam*wi)*r1 + c,  lam = std1/std2
    so build v = wh + lam*wi once, and fold r1/c into the ACT gate ops:
    gates = sigmoid/tanh(v*r1 + c).
Mean/sumsq per row come from fused accum_out on the PSUM->SBUF copies and
Square passes.  sqrt via Heron iteration (ACT Sqrt lives in a different
activation-table set than Sigmoid/Tanh; swapping tables costs ~2.7us).
"""

import os
import sys

import numpy as np

sys.path.insert(0, "/opt/trn_rl_repo")

B, I, H = 16384, 512, 512
N4H = 4 * H
NCORES = 8
BC = B // NCORES            # rows per core
P = 128                     # partitions
MTILES = BC // P            # 16 m-tiles of 128 rows
EPS = 1e-5

# matmul input dtype: "float32r" = full-rate 1 cyc/row fp32 path,
# "float32" = exact but 4 cyc/row, "bfloat16" = fast, lower precision.
MM_DTYPE = os.environ.get("LSTM_MM_DTYPE", "float32r")
HERON_ITERS = 5
CUT = 99   # debug: stop tile body after phase N

_built = {}

RSQRT_MAGIC = 0x5F3759DF
NEWTON_ITERS = 4


def _rsqrt(nc, pool, magic_tile, v, suffix):
    """y ~= 1/sqrt(v) elementwise on tiny [P, w] tiles, using only Pool-legal
    ALU ops (mult/add/sub/shift). Quake seed + NEWTON_ITERS Newton steps."""
    from concourse import mybir

    OP = mybir.AluOpType
    f32 = mybir.dt.float32
    i32 = mybir.dt.int32
    g = nc.gpsimd
    P, w = v.shape

    y = pool.tile([P, w], f32, tag=f"rsq_y{suffix}", name=f"rsq_y{suffix}")
    sh = y[:].bitcast(i32)
    # seed bits = magic - (bits(v) >> 1); shift on DVE (int-exact there)
    nc.vector.tensor_scalar(out=sh, in0=v[:].bitcast(i32), scalar1=1,
                            scalar2=None, op0=OP.arith_shift_right)
    g.tensor_tensor(out=sh, in0=magic_tile[:, 0:w], in1=sh, op=OP.subtract)
    t = pool.tile([P, w], f32, tag=f"rsq_t{suffix}", name=f"rsq_t{suffix}")
    for _ in range(NEWTON_ITERS):
        g.tensor_tensor(out=t[:], in0=y[:], in1=y[:], op=OP.mult)
        g.tensor_tensor(out=t[:], in0=t[:], in1=v[:], op=OP.mult)
        g.tensor_scalar(out=t[:], in0=t[:], scalar1=-0.5, scalar2=1.5,
                        op0=OP.mult, op1=OP.add)
        g.tensor_tensor(out=y[:], in0=y[:], in1=t[:], op=OP.mult)
    return y


def _build(kc_chunks, general):
    """Build the Bass module. kc_chunks: number of 128-row K chunks (4, or 5
    when biases are folded in via the K-extension). general: apply per-column
    gamma/beta tensors (g_hh/g_ih/b_hh/b_ih and g_c/b_c) explicitly."""
    import concourse.bass as bass
    import concourse.bacc as bacc
    import concourse.tile as tile
    from concourse import mybir
    from contextlib import ExitStack

    f32 = mybir.dt.float32
    i32 = mybir.dt.int32
    mmdt = getattr(mybir.dt, MM_DTYPE)
    AF = mybir.ActivationFunctionType
    OP = mybir.AluOpType

    KC = kc_chunks
    KI = KC * P                 # padded contraction size

    nc = bacc.Bacc()

    # Per-core inputs (pre-transposed/padded on host).
    xT = nc.dram_tensor("xT", [KI, BC], mmdt, kind="ExternalInput")
    hT = nc.dram_tensor("hT", [KI, BC], mmdt, kind="ExternalInput")
    c0s = nc.dram_tensor("c0s", [BC, H], f32, kind="ExternalInput")
    w_ih_d = nc.dram_tensor("w_ih", [KI, N4H], mmdt, kind="ExternalInput")
    w_hh_d = nc.dram_tensor("w_hh", [KI, N4H], mmdt, kind="ExternalInput")
    if general:
        # rows: g_hh, b_hh, g_ih, b_ih  (per-column, replicated to 128 parts)
        gb = nc.dram_tensor("gb", [4, N4H], f32, kind="ExternalInput")
        gbc = nc.dram_tensor("gbc", [2, H], f32, kind="ExternalInput")
    h1s = nc.dram_tensor("h1s", [BC, H], f32, kind="ExternalOutput")
    c1s = nc.dram_tensor("c1s", [BC, H], f32, kind="ExternalOutput")

    xT_r = xT[:, :].rearrange("(kc p) (t m) -> t p kc m", p=P, m=P)
    hT_r = hT[:, :].rearrange("(kc p) (t m) -> t p kc m", p=P, m=P)
    w_ih_r = w_ih_d[:, :].rearrange("(kc p) n -> p kc n", p=P)
    w_hh_r = w_hh_d[:, :].rearrange("(kc p) n -> p kc n", p=P)
    c0_r = c0s[:, :].rearrange("(t m) h -> t m h", m=P)
    h1_r = h1s[:, :].rearrange("(t m) h -> t m h", m=P)
    c1_r = c1s[:, :].rearrange("(t m) h -> t m h", m=P)

    with tile.TileContext(nc) as tc, ExitStack() as ctx:
        consts = ctx.enter_context(tc.tile_pool(name="consts", bufs=1))
        xh_pool = ctx.enter_context(tc.tile_pool(name="xh", bufs=2))
        psum = ctx.enter_context(tc.tile_pool(name="psum", bufs=1, space="PSUM"))
        big = ctx.enter_context(tc.tile_pool(name="big", bufs=2))
        gate = ctx.enter_context(tc.tile_pool(name="gate", bufs=2))
        io = ctx.enter_context(tc.tile_pool(name="io", bufs=2))
        small = ctx.enter_context(tc.tile_pool(name="small", bufs=3))

        # Magic-constant tiles for the rsqrt seed.
        magic2 = consts.tile([P, 2], i32, name="magic2")
        nc.vector.memset(magic2[:], RSQRT_MAGIC)
        magic1 = magic2[:, 0:1]

        # Weights resident in SBUF for the whole kernel.
        w_hh_sb = consts.tile([P, KC, N4H], mmdt)
        nc.sync.dma_start(out=w_hh_sb[:], in_=w_hh_r)
        w_ih_sb = consts.tile([P, KC, N4H], mmdt)
        nc.sync.dma_start(out=w_ih_sb[:], in_=w_ih_r)
        if general:
            gb_ap = gb[:, :]
            gb_sb = consts.tile([P, 4, N4H], f32)
            nc.sync.dma_start(
                out=gb_sb[:],
                in_=bass.AP(tensor=gb_ap.tensor, offset=gb_ap.offset,
                            ap=[[0, P], *[list(a) for a in gb_ap.ap]]),
            )
            gbc_ap = gbc[:, :]
            gbc_sb = consts.tile([P, 2, H], f32)
            nc.sync.dma_start(
                out=gbc_sb[:],
                in_=bass.AP(tensor=gbc_ap.tensor, offset=gbc_ap.offset,
                            ap=[[0, P], *[list(a) for a in gbc_ap.ap]]),
            )

        for t in range(MTILES):
            # ---- loads -------------------------------------------------
            xt = xh_pool.tile([P, KC, P], mmdt, tag="xt")
            nc.sync.dma_start(out=xt[:], in_=xT_r[t])
            ht = xh_pool.tile([P, KC, P], mmdt, tag="ht")
            nc.sync.dma_start(out=ht[:], in_=hT_r[t])
            c0t = io.tile([P, H], f32, tag="c0t")
            nc.sync.dma_start(out=c0t[:], in_=c0_r[t])

            # ---- GEMMs into PSUM ---------------------------------------
            # wh = hT.T @ w_hh ; wi = xT.T @ w_ih    (each [128, 2048])
            wh_ps = [psum.tile([P, 2, 512], f32, tag=f"ps{j}", name=f"wh_ps{j}")
                     for j in range(2)]
            wi_ps = [psum.tile([P, 2, 512], f32, tag=f"ps{j + 2}",
                               name=f"wi_ps{j}") for j in range(2)]
            for ps, lhs, w_sb in ((wh_ps, ht, w_hh_sb), (wi_ps, xt, w_ih_sb)):
                for kc in range(KC):
                    for n in range(4):
                        nc.tensor.matmul(
                            ps[n // 2][:, n % 2, :],
                            lhs[:, kc, :],
                            w_sb[:, kc, n * 512:(n + 1) * 512],
                            start=(kc == 0),
                            stop=(kc == KC - 1),
                        )

            # ---- PSUM -> SBUF copies with fused row-sums ---------------
            sums = small.tile([P, 4], f32, tag="sums")    # wh0 wh1 wi0 wi1
            sqs = small.tile([P, 4], f32, tag="sqs")      # wh0 wh1 wi0 wi1 sq
            wh_sb = big.tile([P, N4H], f32, tag="wh_sb")
            for j in range(2):
                nc.scalar.activation(
                    out=wh_sb[:, j * 1024:(j + 1) * 1024],
                    in_=wh_ps[j][:].rearrange("p a b -> p (a b)"),
                    func=AF.Identity,
                    accum_out=sums[:, j:j + 1],
                )
            wi_sb = big.tile([P, N4H], f32, tag="wi_sb")
            for j in range(2):
                nc.vector.tensor_scalar(
                    out=wi_sb[:, j * 1024:(j + 1) * 1024],
                    in0=wi_ps[j][:].rearrange("p a b -> p (a b)"),
                    scalar1=1.0,
                    scalar2=None,
                    op0=OP.mult,
                    op1=OP.add,
                    accum_out=sums[:, 2 + j:3 + j],
                )

            if CUT <= 1:
                nc.sync.dma_start(out=c1_r[t], in_=wh_sb[:, 0:H])
                nc.sync.dma_start(out=h1_r[t], in_=wi_sb[:, 0:H])
                continue
            # ---- row sum-of-squares ------------------------------------
            # NB: tensor_tensor_reduce with in0 == in1 (same AP) hard-faults
            # the device; read one operand from PSUM and one from SBUF.
            scr = big.tile([P, N4H], mybir.dt.bfloat16, tag="scr")
            scr_t = big.tile([P, N4H], mybir.dt.bfloat16, tag="scr_t")
            for j in range(2):
                nc.scalar.activation(
                    out=scr[:, j * 1024:(j + 1) * 1024],
                    in_=wh_sb[:, j * 1024:(j + 1) * 1024], func=AF.Square,
                    accum_out=sqs[:, j:j + 1],
                )
                nc.vector.scalar_tensor_tensor(
                    out=scr_t[:, j * 1024:(j + 1) * 1024],
                    in0=wi_ps[j][:].rearrange("p a b -> p (a b)"),
                    scalar=1.0,
                    in1=wi_sb[:, j * 1024:(j + 1) * 1024],
                    op0=OP.mult, op1=OP.mult,
                    accum_out=sqs[:, 2 + j:3 + j],
                )

            if CUT <= 2:
                nc.sync.dma_start(out=c1_r[t], in_=wh_sb[:, 0:H])
                nc.sync.dma_start(out=h1_r[t], in_=wi_sb[:, 0:H])
                continue
            # ---- stats chain on gpsimd (tiny [P,2] ops) ----------------
            g = nc.gpsimd
            m2t = small.tile([P, 2], f32, tag="m2t")       # row means (wh, wi)
            g.tensor_tensor(out=m2t[:], in0=sums[:, 0::2], in1=sums[:, 1::2],
                            op=OP.add)
            g.tensor_scalar(out=m2t[:], in0=m2t[:], scalar1=1.0 / N4H,
                            scalar2=None, op0=OP.mult)
            msq = small.tile([P, 2], f32, tag="msq")
            g.tensor_tensor(out=msq[:], in0=m2t[:], in1=m2t[:], op=OP.mult)
            sq2 = small.tile([P, 2], f32, tag="sq2")
            g.tensor_tensor(out=sq2[:], in0=sqs[:, 0::2], in1=sqs[:, 1::2],
                            op=OP.add)
            ve = small.tile([P, 2], f32, tag="ve")         # var + eps
            g.tensor_scalar(out=ve[:], in0=sq2[:], scalar1=1.0 / N4H,
                            scalar2=EPS, op0=OP.mult, op1=OP.add)
            g.tensor_tensor(out=ve[:], in0=ve[:], in1=msq[:], op=OP.subtract)
            # rsqrt via Newton y*=1.5-0.5*v*y^2, magic-int seed (mult/add only
            # -- Pool has no divide/sqrt, and ACT's Sqrt is in a different
            # activation-table set than Sigmoid/Tanh).
            rr = _rsqrt(nc, small, magic2, ve, "a")        # 1/sqrt(var+eps)
            std = small.tile([P, 2], f32, tag="std")
            g.tensor_tensor(out=std[:], in0=ve[:], in1=rr[:], op=OP.mult)
            lam = small.tile([P, 1], f32, tag="lam")       # std1/std2
            g.tensor_tensor(out=lam[:], in0=std[:, 0:1], in1=rr[:, 1:2],
                            op=OP.mult)
            # c = -(m1*r1 + m2*r2)
            mr = small.tile([P, 2], f32, tag="mr")
            g.tensor_tensor(out=mr[:], in0=m2t[:], in1=rr[:], op=OP.mult)
            cneg = small.tile([P, 1], f32, tag="cneg")
            g.tensor_tensor(out=cneg[:], in0=mr[:, 0:1], in1=mr[:, 1:2],
                            op=OP.add)
            g.tensor_scalar(out=cneg[:], in0=cneg[:], scalar1=-1.0,
                            scalar2=None, op0=OP.mult)

            if CUT <= 3:
                nc.sync.dma_start(out=c1_r[t], in_=wh_sb[:, 0:H])
                nc.sync.dma_start(out=h1_r[t], in_=wi_sb[:, 0:H])
                continue
            # ---- v = wh + lam*wi  -------------------------------------
            v = big.tile([P, N4H], f32, tag="v")
            nc.vector.scalar_tensor_tensor(
                out=v[:], in0=wi_sb[:], scalar=lam[:], in1=wh_sb[:],
                op0=OP.mult, op1=OP.add,
            )

            r1 = rr[:, 0:1]
            if general:
                # s = (v*r1 + c)*g + b  -> materialize s then plain gates
                sfull = big.tile([P, N4H], f32, tag="sfull")
                nc.scalar.activation(out=sfull[:], in_=v[:], func=AF.Identity,
                                     bias=cneg[:], scale=r1)
                nc.vector.tensor_mul(out=sfull[:], in0=sfull[:],
                                     in1=gb_sb[:, 0, :])
                nc.vector.tensor_add(out=sfull[:], in0=sfull[:],
                                     in1=gb_sb[:, 1, :])
                gate_in, gate_scale, gate_bias = sfull, 1.0, 0.0
            else:
                gate_in, gate_scale, gate_bias = v, r1, cneg[:]

            # ---- gates -------------------------------------------------
            sfio = gate.tile([P, 3 * H], f32, tag="sfio")
            nc.scalar.activation(out=sfio[:], in_=gate_in[:, 0:3 * H],
                                 func=AF.Sigmoid,
                                 bias=gate_bias, scale=gate_scale)
            tg = gate.tile([P, H], f32, tag="tg")
            nc.scalar.activation(out=tg[:], in_=gate_in[:, 3 * H:4 * H],
                                 func=AF.Tanh,
                                 bias=gate_bias, scale=gate_scale)

            if CUT <= 4:
                nc.sync.dma_start(out=c1_r[t], in_=v[:, 0:H])
                nc.sync.dma_start(out=h1_r[t], in_=sfio[:, 0:H])
                continue
            # ---- c1 = sf*c0 + si*tg  (+ fused row-sum) -----------------
            ut = gate.tile([P, H], f32, tag="ut")
            nc.vector.tensor_mul(out=ut[:], in0=sfio[:, 0:H], in1=c0t[:])
            tt = gate.tile([P, H], f32, tag="tt")
            nc.vector.tensor_mul(out=tt[:], in0=sfio[:, H:2 * H], in1=tg[:])
            sc1 = small.tile([P, 1], f32, tag="sc1")
            c1t = io.tile([P, H], f32, tag="c1t")
            nc.vector.scalar_tensor_tensor(
                out=c1t[:], in0=ut[:], scalar=1.0, in1=tt[:],
                op0=OP.mult, op1=OP.add, accum_out=sc1[:],
            )
            nc.sync.dma_start(out=c1_r[t], in_=c1t[:])

            # ---- LN(c1) stats ------------------------------------------
            sqc1 = small.tile([P, 1], f32, tag="sqc1")
            scr2 = gate.tile([P, H], mybir.dt.bfloat16, tag="scr2")
            nc.scalar.activation(out=scr2[:], in_=c1t[:], func=AF.Square,
                                 accum_out=sqc1[:])
            mc = small.tile([P, 1], f32, tag="mc")
            g.tensor_scalar(out=mc[:], in0=sc1[:], scalar1=1.0 / H,
                            scalar2=None, op0=OP.mult)
            mcsq = small.tile([P, 1], f32, tag="mcsq")
            g.tensor_tensor(out=mcsq[:], in0=mc[:], in1=mc[:], op=OP.mult)
            vc = small.tile([P, 1], f32, tag="vc")
            g.tensor_scalar(out=vc[:], in0=sqc1[:], scalar1=1.0 / H,
                            scalar2=EPS, op0=OP.mult, op1=OP.add)
            g.tensor_tensor(out=vc[:], in0=vc[:], in1=mcsq[:], op=OP.subtract)
            rc = _rsqrt(nc, small, magic1, vc, "c")         # 1/sqrt(var_c+eps)
            bcn = small.tile([P, 1], f32, tag="bcn")
            g.tensor_tensor(out=bcn[:], in0=mc[:], in1=rc[:], op=OP.mult)
            g.tensor_scalar(out=bcn[:], in0=bcn[:], scalar1=-1.0, scalar2=None,
                            op0=OP.mult)

            # ---- h1 = so * tanh(LN(c1)) --------------------------------
            th = gate.tile([P, H], f32, tag="th")
            if general:
                lnc = gate.tile([P, H], f32, tag="lnc")
                nc.scalar.activation(out=lnc[:], in_=c1t[:], func=AF.Identity,
                                     bias=bcn[:], scale=rc[:])
                nc.vector.tensor_mul(out=lnc[:], in0=lnc[:], in1=gbc_sb[:, 0, :])
                nc.vector.tensor_add(out=lnc[:], in0=lnc[:], in1=gbc_sb[:, 1, :])
                nc.scalar.activation(out=th[:], in_=lnc[:], func=AF.Tanh)
            else:
                nc.scalar.activation(out=th[:], in_=c1t[:], func=AF.Tanh,
                                     bias=bcn[:], scale=rc[:])
            h1t = io.tile([P, H], f32, tag="h1t")
            nc.vector.tensor_mul(out=h1t[:], in0=sfio[:, 2 * H:3 * H], in1=th[:])
            nc.sync.dma_start(out=h1_r[t], in_=h1t[:])

    if not nc.is_finalized():
        nc.finalize()
    return nc


def _prep_core_inputs(x, h0, c0, w_ih, w_hh, bias_x, bias_h, kc_chunks,
                      general, g_ih, b_ih, g_hh, b_hh, g_c, b_c):
    """Build in_maps for the 8 cores (host-side shard + transpose + pad)."""
    KI = kc_chunks * P
    need_pad = KI != I

    def pad_feat(mT, bias):
        # mT: [I, BC] -> [KI, BC]; appended row of ones picks up the bias row
        out = np.zeros((KI, mT.shape[1]), dtype=np.float32)
        out[:I] = mT
        if need_pad:
            out[I] = 1.0
        return out

    def pad_w(w, bias):
        out = np.zeros((KI, N4H), dtype=np.float32)
        out[:I] = w
        if need_pad:
            out[I] = bias
        return out

    w_ih_p = pad_w(w_ih, bias_x)
    w_hh_p = pad_w(w_hh, bias_h)
    gb = np.stack([g_hh, b_hh, g_ih, b_ih]).astype(np.float32)
    gbc = np.stack([g_c, b_c]).astype(np.float32)

    in_maps = []
    for c in range(NCORES):
        sl = slice(c * BC, (c + 1) * BC)
        m = {
            "xT": pad_feat(np.ascontiguousarray(x[sl].T), bias_x),
            "hT": pad_feat(np.ascontiguousarray(h0[sl].T), bias_h),
            "c0s": np.ascontiguousarray(c0[sl]),
            "w_ih": w_ih_p,
            "w_hh": w_hh_p,
        }
        if general:
            m["gb"] = gb
            m["gbc"] = gbc
        in_maps.append(m)
    return in_maps


def kernel(x, h0, c0, w_ih, w_hh, bias_x, bias_h, g_ih, b_ih, g_hh, b_hh,
           g_c, b_c, _results_hook=None):
    x = np.asarray(x, dtype=np.float32)
    h0 = np.asarray(h0, dtype=np.float32)
    c0 = np.asarray(c0, dtype=np.float32)
    w_ih = np.asarray(w_ih, dtype=np.float32)
    w_hh = np.asarray(w_hh, dtype=np.float32)
    bias_x = np.asarray(bias_x, dtype=np.float32)
    bias_h = np.asarray(bias_h, dtype=np.float32)
    g_ih = np.asarray(g_ih, dtype=np.float32)
    b_ih = np.asarray(b_ih, dtype=np.float32)
    g_hh = np.asarray(g_hh, dtype=np.float32)
    b_hh = np.asarray(b_hh, dtype=np.float32)
    g_c = np.asarray(g_c, dtype=np.float32)
    b_c = np.asarray(b_c, dtype=np.float32)

    have_bias = bool(np.any(bias_x) or np.any(bias_h))
    kc_chunks = 5 if have_bias else 4
    general = not (
        np.all(g_ih == 1) and np.all(g_hh == 1) and np.all(g_c == 1)
        and not np.any(b_ih) and not np.any(b_hh) and not np.any(b_c)
    )

    key = (kc_chunks, general)
    if key not in _built:
        _built[key] = _build(kc_chunks, general)
    nc = _built[key]

    in_maps = _prep_core_inputs(x, h0, c0, w_ih, w_hh, bias_x, bias_h,
                                kc_chunks, general, g_ih, b_ih, g_hh, b_hh,
                                g_c, b_c)

    from concourse.bass_utils import run_bass_kernel_spmd

    res = run_bass_kernel_spmd(nc, in_maps, list(range(NCORES)))
    if _results_hook is not None:
        _results_hook(res)

    h1 = np.concatenate([res.results[c]["h1s"] for c in range(NCORES)], axis=0)
    c1 = np.concatenate([res.results[c]["c1s"] for c in range(NCORES)], axis=0)
    return h1, c1


# revision 23
# speedup vs baseline: 1.2930x; 1.2930x over previous
"""Normalized LSTM cell on 8 Trainium2 NeuronCores (Bass/Tile).

reference:
    wh = h0 @ w_hh + bias_h            # [B, 4H]
    wi = x  @ w_ih + bias_x            # [B, 4H]
    s  = LN(wh)*g_hh + b_hh + LN(wi)*g_ih + b_ih
    f, i, o, g = split(s, 4)
    c1 = sigmoid(f)*c0 + sigmoid(i)*tanh(g)
    h1 = sigmoid(o)*tanh(LN(c1)*g_c + b_c)
    return h1, c1

Sharding: data-parallel over batch, B=16384 -> 2048 rows/core.

Per-core kernel math (fast path: all gammas==1, betas==0, biases==0):
    LN(wh)+LN(wi) = r1*wh + r2*wi + c,   r=1/std, c=-(m1*r1+m2*r2)
                  = (wh + lam*wi)*r1 + c,  lam = std1/std2
    so build v = wh + lam*wi once, and fold r1/c into the ACT gate ops:
    gates = sigmoid/tanh(v*r1 + c).
Mean/sumsq per row come from fused accum_out on the PSUM->SBUF copies and
Square passes.  sqrt via Heron iteration (ACT Sqrt lives in a different
activation-table set than Sigmoid/Tanh; swapping tables costs ~2.7us).
"""

import os
import sys

import numpy as np

sys.path.insert(0, "/opt/trn_rl_repo")

B, I, H = 16384, 512, 512
N4H = 4 * H
NCORES = 8
BC = B // NCORES            # rows per core
P = 128                     # partitions
MTILES = BC // P            # 16 m-tiles of 128 rows
EPS = 1e-5

# matmul input dtype: "float32r" = full-rate 1 cyc/row fp32 path,
# "float32" = exact but 4 cyc/row, "bfloat16" = fast, lower precision.
MM_DTYPE = os.environ.get("LSTM_MM_DTYPE", "float32r")
HERON_ITERS = 5
CUT = 99   # debug: stop tile body after phase N

_built = {}

RSQRT_MAGIC = 0x5F3759DF
NEWTON_ITERS = 2


def _rsqrt(nc, pool, magic_tile, v, suffix):
    """y ~= 1/sqrt(v) elementwise on tiny [P, w] tiles, using only Pool-legal
    ALU ops (mult/add/sub/shift). Quake seed + NEWTON_ITERS Newton steps."""
    from concourse import mybir

    OP = mybir.AluOpType
    f32 = mybir.dt.float32
    i32 = mybir.dt.int32
    g = nc.gpsimd
    P, w = v.shape

    y = pool.tile([P, w], f32, tag=f"rsq_y{suffix}", name=f"rsq_y{suffix}")
    sh = y[:].bitcast(i32)
    # seed bits = magic - (bits(v) >> 1); shift on DVE (int-exact there)
    nc.vector.tensor_scalar(out=sh, in0=v[:].bitcast(i32), scalar1=1,
                            scalar2=None, op0=OP.arith_shift_right)
    V = nc.vector
    V.tensor_tensor(out=sh, in0=magic_tile[:, 0:w], in1=sh, op=OP.subtract)
    t = pool.tile([P, w], f32, tag=f"rsq_t{suffix}", name=f"rsq_t{suffix}")
    for _ in range(NEWTON_ITERS):
        V.tensor_tensor(out=t[:], in0=y[:], in1=y[:], op=OP.mult)
        V.tensor_tensor(out=t[:], in0=t[:], in1=v[:], op=OP.mult)
        V.tensor_scalar(out=t[:], in0=t[:], scalar1=-0.5, scalar2=1.5,
                        op0=OP.mult, op1=OP.add)
        V.tensor_tensor(out=y[:], in0=y[:], in1=t[:], op=OP.mult)
    return y


def _build(kc_chunks, general):
    """Build the Bass module. kc_chunks: number of 128-row K chunks (4, or 5
    when biases are folded in via the K-extension). general: apply per-column
    gamma/beta tensors (g_hh/g_ih/b_hh/b_ih and g_c/b_c) explicitly."""
    import concourse.bass as bass
    import concourse.bacc as bacc
    import concourse.tile as tile
    from concourse import mybir
    from contextlib import ExitStack

    f32 = mybir.dt.float32
    i32 = mybir.dt.int32
    mmdt = getattr(mybir.dt, MM_DTYPE)
    AF = mybir.ActivationFunctionType
    OP = mybir.AluOpType

    KC = kc_chunks
    KI = KC * P                 # padded contraction size

    nc = bacc.Bacc()

    # Per-core inputs (pre-transposed/padded on host).
    xT = nc.dram_tensor("xT", [KI, BC], mmdt, kind="ExternalInput")
    hT = nc.dram_tensor("hT", [KI, BC], mmdt, kind="ExternalInput")
    c0s = nc.dram_tensor("c0s", [BC, H], f32, kind="ExternalInput")
    w_ih_d = nc.dram_tensor("w_ih", [KI, N4H], mmdt, kind="ExternalInput")
    w_hh_d = nc.dram_tensor("w_hh", [KI, N4H], mmdt, kind="ExternalInput")
    if general:
        # rows: g_hh, b_hh, g_ih, b_ih  (per-column, replicated to 128 parts)
        gb = nc.dram_tensor("gb", [4, N4H], f32, kind="ExternalInput")
        gbc = nc.dram_tensor("gbc", [2, H], f32, kind="ExternalInput")
    h1s = nc.dram_tensor("h1s", [BC, H], f32, kind="ExternalOutput")
    c1s = nc.dram_tensor("c1s", [BC, H], f32, kind="ExternalOutput")

    xT_r = xT[:, :].rearrange("(kc p) (t m) -> t p kc m", p=P, m=P)
    hT_r = hT[:, :].rearrange("(kc p) (t m) -> t p kc m", p=P, m=P)
    w_ih_r = w_ih_d[:, :].rearrange("(kc p) n -> p kc n", p=P)
    w_hh_r = w_hh_d[:, :].rearrange("(kc p) n -> p kc n", p=P)
    c0_r = c0s[:, :].rearrange("(t m) h -> t m h", m=P)
    h1_r = h1s[:, :].rearrange("(t m) h -> t m h", m=P)
    c1_r = c1s[:, :].rearrange("(t m) h -> t m h", m=P)

    with tile.TileContext(nc) as tc, ExitStack() as ctx:
        consts = ctx.enter_context(tc.tile_pool(name="consts", bufs=1))
        xh_pool = ctx.enter_context(tc.tile_pool(name="xh", bufs=2))
        psum = ctx.enter_context(tc.tile_pool(name="psum", bufs=1, space="PSUM"))
        big = ctx.enter_context(tc.tile_pool(name="big", bufs=2))
        gate = ctx.enter_context(tc.tile_pool(name="gate", bufs=2))
        io = ctx.enter_context(tc.tile_pool(name="io", bufs=2))
        small = ctx.enter_context(tc.tile_pool(name="small", bufs=3))

        # Magic-constant tiles for the rsqrt seed.
        magic2 = consts.tile([P, 2], i32, name="magic2")
        nc.vector.memset(magic2[:], RSQRT_MAGIC)
        magic1 = magic2[:, 0:1]

        # Weights resident in SBUF for the whole kernel.
        w_hh_sb = consts.tile([P, KC, N4H], mmdt)
        nc.sync.dma_start(out=w_hh_sb[:], in_=w_hh_r)
        w_ih_sb = consts.tile([P, KC, N4H], mmdt)
        nc.sync.dma_start(out=w_ih_sb[:], in_=w_ih_r)
        if general:
            gb_ap = gb[:, :]
            gb_sb = consts.tile([P, 4, N4H], f32)
            nc.sync.dma_start(
                out=gb_sb[:],
                in_=bass.AP(tensor=gb_ap.tensor, offset=gb_ap.offset,
                            ap=[[0, P], *[list(a) for a in gb_ap.ap]]),
            )
            gbc_ap = gbc[:, :]
            gbc_sb = consts.tile([P, 2, H], f32)
            nc.sync.dma_start(
                out=gbc_sb[:],
                in_=bass.AP(tensor=gbc_ap.tensor, offset=gbc_ap.offset,
                            ap=[[0, P], *[list(a) for a in gbc_ap.ap]]),
            )

        for t in range(MTILES):
            # ---- loads -------------------------------------------------
            xt = xh_pool.tile([P, KC, P], mmdt, tag="xt")
            nc.sync.dma_start(out=xt[:], in_=xT_r[t])
            ht = xh_pool.tile([P, KC, P], mmdt, tag="ht")
            nc.sync.dma_start(out=ht[:], in_=hT_r[t])
            c0t = io.tile([P, H], f32, tag="c0t")
            nc.sync.dma_start(out=c0t[:], in_=c0_r[t])

            # ---- GEMMs into PSUM ---------------------------------------
            # wh = hT.T @ w_hh ; wi = xT.T @ w_ih    (each [128, 2048])
            wh_ps = psum.tile([P, 4, 512], f32, tag="ps_wh")
            wi_ps = [psum.tile([P, 2, 512], f32, tag=f"ps_wi{j}",
                               name=f"wi_ps{j}") for j in range(2)]
            for kc in range(KC):
                for n in range(4):
                    nc.tensor.matmul(
                        wh_ps[:, n, :], ht[:, kc, :],
                        w_hh_sb[:, kc, n * 512:(n + 1) * 512],
                        start=(kc == 0), stop=(kc == KC - 1))
            for kc in range(KC):
                for n in range(4):
                    nc.tensor.matmul(
                        wi_ps[n // 2][:, n % 2, :], xt[:, kc, :],
                        w_ih_sb[:, kc, n * 512:(n + 1) * 512],
                        start=(kc == 0), stop=(kc == KC - 1))

            # ---- wh: PSUM->SBUF copy + Sum(wh), then Sum(wh^2) on ACT --
            sums = small.tile([P, 2], f32, tag="sums")    # Sum(wh), Sum(wh^2)
            wh_sb = big.tile([P, N4H], f32, tag="wh_sb")
            nc.scalar.activation(
                out=wh_sb[:], in_=wh_ps[:].rearrange("p a b -> p (a b)"),
                func=AF.Identity, accum_out=sums[:, 0:1])
            scr = big.tile([P, N4H], mybir.dt.bfloat16, tag="scr")
            nc.scalar.activation(
                out=scr[:], in_=wh_sb[:], func=AF.Square,
                accum_out=sums[:, 1:2])

            if CUT <= 1 or CUT == 2:
                nc.sync.dma_start(out=c1_r[t], in_=wh_sb[:, 0:H])
                nc.sync.dma_start(out=h1_r[t], in_=wh_sb[:, H:2 * H])
                continue

            # ---- wi: bn_stats straight off PSUM ------------------------
            bst = small.tile([P, 4, 6], f32, tag="bst")
            for j in range(2):
                for i in range(2):
                    nc.vector.bn_stats(out=bst[:, 2 * j + i, :],
                                       in_=wi_ps[j][:, i, :])
            mv = small.tile([P, 2], f32, tag="mv")        # mean(wi), var(wi)
            nc.vector.bn_aggr(out=mv[:], in_=bst[:])

            # ---- stats chain (tiny DVE ops) ----------------------------
            V = nc.vector
            m2t = small.tile([P, 2], f32, tag="m2t")      # m1, m2
            V.tensor_scalar(out=m2t[:, 0:1], in0=sums[:, 0:1],
                            scalar1=1.0 / N4H, scalar2=None, op0=OP.mult)
            V.tensor_copy(out=m2t[:, 1:2], in_=mv[:, 0:1])
            ve = small.tile([P, 2], f32, tag="ve")        # var+eps (wh, wi)
            V.tensor_scalar(out=ve[:, 0:1], in0=sums[:, 1:2],
                            scalar1=1.0 / N4H, scalar2=EPS,
                            op0=OP.mult, op1=OP.add)
            msq = small.tile([P, 1], f32, tag="msq")
            V.tensor_tensor(out=msq[:], in0=m2t[:, 0:1], in1=m2t[:, 0:1],
                            op=OP.mult)
            V.tensor_tensor(out=ve[:, 0:1], in0=ve[:, 0:1], in1=msq[:],
                            op=OP.subtract)
            V.tensor_scalar(out=ve[:, 1:2], in0=mv[:, 1:2], scalar1=EPS,
                            scalar2=None, op0=OP.add)
            rr = _rsqrt(nc, small, magic2, ve, "a")       # r1, r2
            lam = small.tile([P, 1], f32, tag="lam")      # std1/std2
            V.tensor_tensor(out=lam[:], in0=ve[:, 0:1], in1=rr[:, 0:1],
                            op=OP.mult)
            V.tensor_tensor(out=lam[:], in0=lam[:], in1=rr[:, 1:2],
                            op=OP.mult)
            mr = small.tile([P, 2], f32, tag="mr")
            V.tensor_tensor(out=mr[:], in0=m2t[:], in1=rr[:], op=OP.mult)
            cneg = small.tile([P, 1], f32, tag="cneg")
            V.tensor_tensor(out=cneg[:], in0=mr[:, 0:1], in1=mr[:, 1:2],
                            op=OP.add)
            V.tensor_scalar(out=cneg[:], in0=cneg[:], scalar1=-1.0,
                            scalar2=None, op0=OP.mult)

            if CUT == 3:
                nc.sync.dma_start(out=c1_r[t], in_=wh_sb[:, 0:H])
                nc.sync.dma_start(out=h1_r[t], in_=wh_sb[:, H:2 * H])
                continue

            # ---- v = wh + lam*wi (wi read straight from PSUM) ----------
            v = big.tile([P, N4H], f32, tag="v")
            for j in range(2):
                nc.vector.scalar_tensor_tensor(
                    out=v[:, j * 1024:(j + 1) * 1024],
                    in0=wi_ps[j][:].rearrange("p a b -> p (a b)"),
                    scalar=lam[:],
                    in1=wh_sb[:, j * 1024:(j + 1) * 1024],
                    op0=OP.mult, op1=OP.add)

            r1 = rr[:, 0:1]
            if general:
                # s = (v*r1 + c)*g + b  -> materialize s then plain gates
                sfull = big.tile([P, N4H], f32, tag="sfull")
                nc.scalar.activation(out=sfull[:], in_=v[:], func=AF.Identity,
                                     bias=cneg[:], scale=r1)
                nc.vector.tensor_mul(out=sfull[:], in0=sfull[:],
                                     in1=gb_sb[:, 0, :])
                nc.vector.tensor_add(out=sfull[:], in0=sfull[:],
                                     in1=gb_sb[:, 1, :])
                gate_in, gate_scale, gate_bias = sfull, 1.0, 0.0
            else:
                gate_in, gate_scale, gate_bias = v, r1, cneg[:]

            # ---- gates -------------------------------------------------
            sfio = gate.tile([P, 3 * H], f32, tag="sfio")
            nc.scalar.activation(out=sfio[:], in_=gate_in[:, 0:3 * H],
                                 func=AF.Sigmoid,
                                 bias=gate_bias, scale=gate_scale)
            tg = gate.tile([P, H], f32, tag="tg")
            nc.scalar.activation(out=tg[:], in_=gate_in[:, 3 * H:4 * H],
                                 func=AF.Tanh,
                                 bias=gate_bias, scale=gate_scale)

            if CUT == 4:
                nc.sync.dma_start(out=c1_r[t], in_=v[:, 0:H])
                nc.sync.dma_start(out=h1_r[t], in_=sfio[:, 0:H])
                continue

            # ---- c1 = sf*c0 + si*tg  (muls on Pool, add+rowsum on DVE) -
            gp = nc.gpsimd
            ut = gate.tile([P, H], f32, tag="ut")
            gp.tensor_tensor(out=ut[:], in0=sfio[:, 0:H], in1=c0t[:],
                             op=OP.mult)
            tt = gate.tile([P, H], f32, tag="tt")
            gp.tensor_tensor(out=tt[:], in0=sfio[:, H:2 * H], in1=tg[:],
                             op=OP.mult)
            sc1 = small.tile([P, 1], f32, tag="sc1")
            c1t = io.tile([P, H], f32, tag="c1t")
            nc.vector.scalar_tensor_tensor(
                out=c1t[:], in0=ut[:], scalar=1.0, in1=tt[:],
                op0=OP.mult, op1=OP.add, accum_out=sc1[:])
            nc.sync.dma_start(out=c1_r[t], in_=c1t[:])

            # ---- LN(c1) stats ------------------------------------------
            sqc1 = small.tile([P, 1], f32, tag="sqc1")
            scr2 = gate.tile([P, H], mybir.dt.bfloat16, tag="scr2")
            nc.scalar.activation(out=scr2[:], in_=c1t[:], func=AF.Square,
                                 accum_out=sqc1[:])
            mc = small.tile([P, 1], f32, tag="mc")
            V.tensor_scalar(out=mc[:], in0=sc1[:], scalar1=1.0 / H,
                            scalar2=None, op0=OP.mult)
            mcsq = small.tile([P, 1], f32, tag="mcsq")
            V.tensor_tensor(out=mcsq[:], in0=mc[:], in1=mc[:], op=OP.mult)
            vc = small.tile([P, 1], f32, tag="vc")
            V.tensor_scalar(out=vc[:], in0=sqc1[:], scalar1=1.0 / H,
                            scalar2=EPS, op0=OP.mult, op1=OP.add)
            V.tensor_tensor(out=vc[:], in0=vc[:], in1=mcsq[:], op=OP.subtract)
            rc = _rsqrt(nc, small, magic1, vc, "c")
            bcn = small.tile([P, 1], f32, tag="bcn")
            V.tensor_tensor(out=bcn[:], in0=mc[:], in1=rc[:], op=OP.mult)
            V.tensor_scalar(out=bcn[:], in0=bcn[:], scalar1=-1.0, scalar2=None,
                            op0=OP.mult)

            # ---- h1 = so * tanh(LN(c1)) --------------------------------
            th = gate.tile([P, H], f32, tag="th")
            if general:
                lnc = gate.tile([P, H], f32, tag="lnc")
                nc.scalar.activation(out=lnc[:], in_=c1t[:], func=AF.Identity,
                                     bias=bcn[:], scale=rc[:])
                nc.vector.tensor_mul(out=lnc[:], in0=lnc[:], in1=gbc_sb[:, 0, :])
                nc.vector.tensor_add(out=lnc[:], in0=lnc[:], in1=gbc_sb[:, 1, :])
                nc.scalar.activation(out=th[:], in_=lnc[:], func=AF.Tanh)
            else:
                nc.scalar.activation(out=th[:], in_=c1t[:], func=AF.Tanh,
                                     bias=bcn[:], scale=rc[:])
            h1t = io.tile([P, H], f32, tag="h1t")
            gp.tensor_tensor(out=h1t[:], in0=sfio[:, 2 * H:3 * H], in1=th[:],
                             op=OP.mult)
            nc.sync.dma_start(out=h1_r[t], in_=h1t[:])

    if not nc.is_finalized():
        nc.finalize()
    return nc


def _prep_core_inputs(x, h0, c0, w_ih, w_hh, bias_x, bias_h, kc_chunks,
                      general, g_ih, b_ih, g_hh, b_hh, g_c, b_c):
    """Build in_maps for the 8 cores (host-side shard + transpose + pad)."""
    KI = kc_chunks * P
    need_pad = KI != I

    def pad_feat(mT, bias):
        # mT: [I, BC] -> [KI, BC]; appended row of ones picks up the bias row
        out = np.zeros((KI, mT.shape[1]), dtype=np.float32)
        out[:I] = mT
        if need_pad:
            out[I] = 1.0
        return out

    def pad_w(w, bias):
        out = np.zeros((KI, N4H), dtype=np.float32)
        out[:I] = w
        if need_pad:
            out[I] = bias
        return out

    w_ih_p = pad_w(w_ih, bias_x)
    w_hh_p = pad_w(w_hh, bias_h)
    gb = np.stack([g_hh, b_hh, g_ih, b_ih]).astype(np.float32)
    gbc = np.stack([g_c, b_c]).astype(np.float32)

    in_maps = []
    for c in range(NCORES):
        sl = slice(c * BC, (c + 1) * BC)
        m = {
            "xT": pad_feat(np.ascontiguousarray(x[sl].T), bias_x),
            "hT": pad_feat(np.ascontiguousarray(h0[sl].T), bias_h),
            "c0s": np.ascontiguousarray(c0[sl]),
            "w_ih": w_ih_p,
            "w_hh": w_hh_p,
        }
        if general:
            m["gb"] = gb
            m["gbc"] = gbc
        in_maps.append(m)
    return in_maps


def kernel(x, h0, c0, w_ih, w_hh, bias_x, bias_h, g_ih, b_ih, g_hh, b_hh,
           g_c, b_c, _results_hook=None):
    x = np.asarray(x, dtype=np.float32)
    h0 = np.asarray(h0, dtype=np.float32)
    c0 = np.asarray(c0, dtype=np.float32)
    w_ih = np.asarray(w_ih, dtype=np.float32)
    w_hh = np.asarray(w_hh, dtype=np.float32)
    bias_x = np.asarray(bias_x, dtype=np.float32)
    bias_h = np.asarray(bias_h, dtype=np.float32)
    g_ih = np.asarray(g_ih, dtype=np.float32)
    b_ih = np.asarray(b_ih, dtype=np.float32)
    g_hh = np.asarray(g_hh, dtype=np.float32)
    b_hh = np.asarray(b_hh, dtype=np.float32)
    g_c = np.asarray(g_c, dtype=np.float32)
    b_c = np.asarray(b_c, dtype=np.float32)

    have_bias = bool(np.any(bias_x) or np.any(bias_h))
    kc_chunks = 5 if have_bias else 4
    general = not (
        np.all(g_ih == 1) and np.all(g_hh == 1) and np.all(g_c == 1)
        and not np.any(b_ih) and not np.any(b_hh) and not np.any(b_c)
    )

    key = (kc_chunks, general)
    if key not in _built:
        _built[key] = _build(kc_chunks, general)
    nc = _built[key]

    in_maps = _prep_core_inputs(x, h0, c0, w_ih, w_hh, bias_x, bias_h,
                                kc_chunks, general, g_ih, b_ih, g_hh, b_hh,
                                g_c, b_c)

    from concourse.bass_utils import run_bass_kernel_spmd

    res = run_bass_kernel_spmd(nc, in_maps, list(range(NCORES)))
    if _results_hook is not None:
        _results_hook(res)

    h1 = np.concatenate([res.results[c]["h1s"] for c in range(NCORES)], axis=0)
    c1 = np.concatenate([res.results[c]["c1s"] for c in range(NCORES)], axis=0)
    return h1, c1


# revision 24
# speedup vs baseline: 1.5017x; 1.1615x over previous
"""Normalized LSTM cell on 8 Trainium2 NeuronCores (Bass/Tile).

reference:
    wh = h0 @ w_hh + bias_h            # [B, 4H]
    wi = x  @ w_ih + bias_x            # [B, 4H]
    s  = LN(wh)*g_hh + b_hh + LN(wi)*g_ih + b_ih
    f, i, o, g = split(s, 4)
    c1 = sigmoid(f)*c0 + sigmoid(i)*tanh(g)
    h1 = sigmoid(o)*tanh(LN(c1)*g_c + b_c)
    return h1, c1

Sharding: data-parallel over batch, B=16384 -> 2048 rows/core.

Per-core kernel math (fast path: all gammas==1, betas==0, biases==0):
    LN(wh)+LN(wi) = r1*wh + r2*wi + c,   r=1/std, c=-(m1*r1+m2*r2)
                  = (wh + lam*wi)*r1 + c,  lam = std1/std2
    so build v = wh + lam*wi once, and fold r1/c into the ACT gate ops:
    gates = sigmoid/tanh(v*r1 + c).
Mean/sumsq per row come from fused accum_out on the PSUM->SBUF copies and
Square passes.  sqrt via Heron iteration (ACT Sqrt lives in a different
activation-table set than Sigmoid/Tanh; swapping tables costs ~2.7us).
"""

import os
import sys

import numpy as np

sys.path.insert(0, "/opt/trn_rl_repo")

B, I, H = 16384, 512, 512
N4H = 4 * H
NCORES = 8
BC = B // NCORES            # rows per core
P = 128                     # partitions
MTILES = BC // P            # 16 m-tiles of 128 rows
EPS = 1e-5

# matmul input dtype: "float32r" = full-rate 1 cyc/row fp32 path,
# "float32" = exact but 4 cyc/row, "bfloat16" = fast, lower precision.
MM_DTYPE = os.environ.get("LSTM_MM_DTYPE", "bfloat16")
HERON_ITERS = 5
CUT = 99   # debug: stop tile body after phase N

_built = {}

RSQRT_MAGIC = 0x5F3759DF
NEWTON_ITERS = int(os.environ.get("LSTM_NEWTON", "1"))


def _rsqrt(nc, pool, magic_tile, v, suffix):
    """y ~= 1/sqrt(v) elementwise on tiny [P, w] tiles, using only Pool-legal
    ALU ops (mult/add/sub/shift). Quake seed + NEWTON_ITERS Newton steps."""
    from concourse import mybir

    OP = mybir.AluOpType
    f32 = mybir.dt.float32
    i32 = mybir.dt.int32
    g = nc.gpsimd
    P, w = v.shape

    y = pool.tile([P, w], f32, tag=f"rsq_y{suffix}", name=f"rsq_y{suffix}")
    sh = y[:].bitcast(i32)
    # seed bits = magic - (bits(v) >> 1); shift on DVE (int-exact there)
    nc.vector.tensor_scalar(out=sh, in0=v[:].bitcast(i32), scalar1=1,
                            scalar2=None, op0=OP.arith_shift_right)
    V = nc.vector
    V.tensor_tensor(out=sh, in0=magic_tile[:, 0:w], in1=sh, op=OP.subtract)
    t = pool.tile([P, w], f32, tag=f"rsq_t{suffix}", name=f"rsq_t{suffix}")
    for _ in range(NEWTON_ITERS):
        V.tensor_tensor(out=t[:], in0=y[:], in1=y[:], op=OP.mult)
        V.tensor_tensor(out=t[:], in0=t[:], in1=v[:], op=OP.mult)
        V.tensor_scalar(out=t[:], in0=t[:], scalar1=-0.5, scalar2=1.5,
                        op0=OP.mult, op1=OP.add)
        V.tensor_tensor(out=y[:], in0=y[:], in1=t[:], op=OP.mult)
    return y


def _build(kc_chunks, general):
    """Build the Bass module. kc_chunks: number of 128-row K chunks (4, or 5
    when biases are folded in via the K-extension). general: apply per-column
    gamma/beta tensors (g_hh/g_ih/b_hh/b_ih and g_c/b_c) explicitly."""
    import concourse.bass as bass
    import concourse.bacc as bacc
    import concourse.tile as tile
    from concourse import mybir
    from contextlib import ExitStack

    f32 = mybir.dt.float32
    i32 = mybir.dt.int32
    mmdt = getattr(mybir.dt, MM_DTYPE)
    AF = mybir.ActivationFunctionType
    OP = mybir.AluOpType

    KC = kc_chunks
    KI = KC * P                 # padded contraction size

    nc = bacc.Bacc()

    # Per-core inputs (pre-transposed/padded on host).
    xT = nc.dram_tensor("xT", [KI, BC], mmdt, kind="ExternalInput")
    hT = nc.dram_tensor("hT", [KI, BC], mmdt, kind="ExternalInput")
    c0s = nc.dram_tensor("c0s", [BC, H], f32, kind="ExternalInput")
    w_ih_d = nc.dram_tensor("w_ih", [KI, N4H], mmdt, kind="ExternalInput")
    w_hh_d = nc.dram_tensor("w_hh", [KI, N4H], mmdt, kind="ExternalInput")
    if general:
        # rows: g_hh, b_hh, g_ih, b_ih  (per-column, replicated to 128 parts)
        gb = nc.dram_tensor("gb", [4, N4H], f32, kind="ExternalInput")
        gbc = nc.dram_tensor("gbc", [2, H], f32, kind="ExternalInput")
    h1s = nc.dram_tensor("h1s", [BC, H], f32, kind="ExternalOutput")
    c1s = nc.dram_tensor("c1s", [BC, H], f32, kind="ExternalOutput")

    xT_r = xT[:, :].rearrange("(kc p) (t m) -> t p kc m", p=P, m=P)
    hT_r = hT[:, :].rearrange("(kc p) (t m) -> t p kc m", p=P, m=P)
    w_ih_r = w_ih_d[:, :].rearrange("(kc p) n -> p kc n", p=P)
    w_hh_r = w_hh_d[:, :].rearrange("(kc p) n -> p kc n", p=P)
    c0_r = c0s[:, :].rearrange("(t m) h -> t m h", m=P)
    h1_r = h1s[:, :].rearrange("(t m) h -> t m h", m=P)
    c1_r = c1s[:, :].rearrange("(t m) h -> t m h", m=P)

    with tile.TileContext(nc) as tc, ExitStack() as ctx:
        consts = ctx.enter_context(tc.tile_pool(name="consts", bufs=1))
        xh_pool = ctx.enter_context(tc.tile_pool(name="xh", bufs=2))
        psum = ctx.enter_context(tc.tile_pool(name="psum", bufs=1, space="PSUM"))
        big = ctx.enter_context(tc.tile_pool(name="big", bufs=2))
        gate = ctx.enter_context(tc.tile_pool(name="gate", bufs=2))
        io = ctx.enter_context(tc.tile_pool(name="io", bufs=2))
        small = ctx.enter_context(tc.tile_pool(name="small", bufs=3))

        # Magic-constant tiles for the rsqrt seed.
        magic2 = consts.tile([P, 2], i32, name="magic2")
        nc.vector.memset(magic2[:], RSQRT_MAGIC)
        magic1 = magic2[:, 0:1]
        inv4h2 = consts.tile([P, 1], f32, name="inv4h2")
        nc.vector.memset(inv4h2[:], 1.0 / (N4H * N4H))
        invh2 = consts.tile([P, 1], f32, name="invh2")
        nc.vector.memset(invh2[:], 1.0 / (H * H))
        neginvh = consts.tile([P, 1], f32, name="neginvh")
        nc.vector.memset(neginvh[:], -1.0 / H)

        # Weights resident in SBUF for the whole kernel.
        w_hh_sb = consts.tile([P, KC, N4H], mmdt)
        nc.sync.dma_start(out=w_hh_sb[:], in_=w_hh_r)
        w_ih_sb = consts.tile([P, KC, N4H], mmdt)
        nc.sync.dma_start(out=w_ih_sb[:], in_=w_ih_r)
        if general:
            gb_ap = gb[:, :]
            gb_sb = consts.tile([P, 4, N4H], f32)
            nc.sync.dma_start(
                out=gb_sb[:],
                in_=bass.AP(tensor=gb_ap.tensor, offset=gb_ap.offset,
                            ap=[[0, P], *[list(a) for a in gb_ap.ap]]),
            )
            gbc_ap = gbc[:, :]
            gbc_sb = consts.tile([P, 2, H], f32)
            nc.sync.dma_start(
                out=gbc_sb[:],
                in_=bass.AP(tensor=gbc_ap.tensor, offset=gbc_ap.offset,
                            ap=[[0, P], *[list(a) for a in gbc_ap.ap]]),
            )

        for t in range(MTILES):
            # ---- loads -------------------------------------------------
            xt = xh_pool.tile([P, KC, P], mmdt, tag="xt")
            nc.sync.dma_start(out=xt[:], in_=xT_r[t])
            ht = xh_pool.tile([P, KC, P], mmdt, tag="ht")
            nc.sync.dma_start(out=ht[:], in_=hT_r[t])
            c0t = io.tile([P, H], f32, tag="c0t")
            nc.sync.dma_start(out=c0t[:], in_=c0_r[t])

            # ---- GEMMs into PSUM ---------------------------------------
            # wh = hT.T @ w_hh ; wi = xT.T @ w_ih    (each [128, 2048])
            wh_ps = psum.tile([P, 4, 512], f32, tag="ps_wh")
            wi_ps = [psum.tile([P, 2, 512], f32, tag=f"ps_wi{j}",
                               name=f"wi_ps{j}") for j in range(2)]
            for kc in range(KC):
                for n in range(4):
                    nc.tensor.matmul(
                        wh_ps[:, n, :], ht[:, kc, :],
                        w_hh_sb[:, kc, n * 512:(n + 1) * 512],
                        start=(kc == 0), stop=(kc == KC - 1))
            for kc in range(KC):
                for n in range(4):
                    nc.tensor.matmul(
                        wi_ps[n // 2][:, n % 2, :], xt[:, kc, :],
                        w_ih_sb[:, kc, n * 512:(n + 1) * 512],
                        start=(kc == 0), stop=(kc == KC - 1))

            # ---- wh: PSUM->SBUF copy + Sum(wh), then Sum(wh^2) on ACT --
            sums = small.tile([P, 2], f32, tag="sums")    # Sum(wh), Sum(wh^2)
            wh_sb = big.tile([P, N4H], f32, tag="wh_sb")
            nc.scalar.activation(
                out=wh_sb[:], in_=wh_ps[:].rearrange("p a b -> p (a b)"),
                func=AF.Identity, accum_out=sums[:, 0:1])
            scr = big.tile([P, N4H], mybir.dt.bfloat16, tag="scr")
            nc.scalar.activation(
                out=scr[:], in_=wh_sb[:], func=AF.Square,
                accum_out=sums[:, 1:2])

            if CUT <= 1 or CUT == 2:
                nc.sync.dma_start(out=c1_r[t], in_=wh_sb[:, 0:H])
                nc.sync.dma_start(out=h1_r[t], in_=wh_sb[:, H:2 * H])
                continue

            # ---- wi: bn_stats straight off PSUM ------------------------
            bst = small.tile([P, 4, 6], f32, tag="bst")
            for j in range(2):
                for i in range(2):
                    nc.vector.bn_stats(out=bst[:, 2 * j + i, :],
                                       in_=wi_ps[j][:, i, :])
            mv = small.tile([P, 2], f32, tag="mv")        # mean(wi), var(wi)
            nc.vector.bn_aggr(out=mv[:], in_=bst[:])

            # ---- stats chain (fused tiny DVE ops) ----------------------
            V = nc.vector
            ve = small.tile([P, 2], f32, tag="ve")        # var+eps (wh, wi)
            V.tensor_scalar(out=ve[:, 0:1], in0=sums[:, 1:2],
                            scalar1=1.0 / N4H, scalar2=EPS,
                            op0=OP.mult, op1=OP.add)
            msq = small.tile([P, 1], f32, tag="msq")      # m1^2 = (S/N)^2
            V.scalar_tensor_tensor(out=msq[:], in0=sums[:, 0:1],
                                   scalar=sums[:, 0:1], in1=inv4h2[:],
                                   op0=OP.mult, op1=OP.mult)
            V.tensor_tensor(out=ve[:, 0:1], in0=ve[:, 0:1], in1=msq[:],
                            op=OP.subtract)
            V.tensor_scalar(out=ve[:, 1:2], in0=mv[:, 1:2], scalar1=EPS,
                            scalar2=None, op0=OP.add)
            rr = _rsqrt(nc, small, magic2, ve, "a")       # r1, r2
            lam = small.tile([P, 1], f32, tag="lam")      # std1/std2 = ve0*r1*r2
            V.scalar_tensor_tensor(out=lam[:], in0=ve[:, 0:1],
                                   scalar=rr[:, 0:1], in1=rr[:, 1:2],
                                   op0=OP.mult, op1=OP.mult)
            mr = small.tile([P, 1], f32, tag="mr")        # m1*r1 = S*(1/N)*r1
            V.scalar_tensor_tensor(out=mr[:], in0=sums[:, 0:1],
                                   scalar=1.0 / N4H, in1=rr[:, 0:1],
                                   op0=OP.mult, op1=OP.mult)
            m2r = small.tile([P, 1], f32, tag="m2r")      # m2*r2 + m1*r1
            V.scalar_tensor_tensor(out=m2r[:], in0=mv[:, 0:1],
                                   scalar=rr[:, 1:2], in1=mr[:],
                                   op0=OP.mult, op1=OP.add)
            cneg = small.tile([P, 1], f32, tag="cneg")
            V.tensor_scalar(out=cneg[:], in0=m2r[:], scalar1=-1.0,
                            scalar2=None, op0=OP.mult)

            if CUT == 3:
                nc.sync.dma_start(out=c1_r[t], in_=wh_sb[:, 0:H])
                nc.sync.dma_start(out=h1_r[t], in_=wh_sb[:, H:2 * H])
                continue

            # ---- v = wh + lam*wi (wi read straight from PSUM) ----------
            v = big.tile([P, N4H], f32, tag="v")
            for j in range(2):
                nc.vector.scalar_tensor_tensor(
                    out=v[:, j * 1024:(j + 1) * 1024],
                    in0=wi_ps[j][:].rearrange("p a b -> p (a b)"),
                    scalar=lam[:],
                    in1=wh_sb[:, j * 1024:(j + 1) * 1024],
                    op0=OP.mult, op1=OP.add)

            r1 = rr[:, 0:1]
            if general:
                # s = (v*r1 + c)*g + b  -> materialize s then plain gates
                sfull = big.tile([P, N4H], f32, tag="sfull")
                nc.scalar.activation(out=sfull[:], in_=v[:], func=AF.Identity,
                                     bias=cneg[:], scale=r1)
                nc.vector.tensor_mul(out=sfull[:], in0=sfull[:],
                                     in1=gb_sb[:, 0, :])
                nc.vector.tensor_add(out=sfull[:], in0=sfull[:],
                                     in1=gb_sb[:, 1, :])
                gate_in, gate_scale, gate_bias = sfull, 1.0, 0.0
            else:
                gate_in, gate_scale, gate_bias = v, r1, cneg[:]

            # ---- gates -------------------------------------------------
            sfio = gate.tile([P, 3 * H], f32, tag="sfio")
            nc.scalar.activation(out=sfio[:], in_=gate_in[:, 0:3 * H],
                                 func=AF.Sigmoid,
                                 bias=gate_bias, scale=gate_scale)
            tg = gate.tile([P, H], f32, tag="tg")
            nc.scalar.activation(out=tg[:], in_=gate_in[:, 3 * H:4 * H],
                                 func=AF.Tanh,
                                 bias=gate_bias, scale=gate_scale)

            if CUT == 4:
                nc.sync.dma_start(out=c1_r[t], in_=v[:, 0:H])
                nc.sync.dma_start(out=h1_r[t], in_=sfio[:, 0:H])
                continue

            # ---- c1 = sf*c0 + si*tg  (muls on Pool, add+rowsum on DVE) -
            gp = nc.gpsimd
            ut = gate.tile([P, H], f32, tag="ut")
            gp.tensor_tensor(out=ut[:], in0=sfio[:, 0:H], in1=c0t[:],
                             op=OP.mult)
            tt = gate.tile([P, H], f32, tag="tt")
            gp.tensor_tensor(out=tt[:], in0=sfio[:, H:2 * H], in1=tg[:],
                             op=OP.mult)
            sc1 = small.tile([P, 1], f32, tag="sc1")
            c1t = io.tile([P, H], f32, tag="c1t")
            nc.vector.scalar_tensor_tensor(
                out=c1t[:], in0=ut[:], scalar=1.0, in1=tt[:],
                op0=OP.mult, op1=OP.add, accum_out=sc1[:])
            nc.sync.dma_start(out=c1_r[t], in_=c1t[:])

            # ---- LN(c1) stats ------------------------------------------
            sqc1 = small.tile([P, 1], f32, tag="sqc1")
            scr2 = gate.tile([P, H], mybir.dt.bfloat16, tag="scr2")
            nc.scalar.activation(out=scr2[:], in_=c1t[:], func=AF.Square,
                                 accum_out=sqc1[:])
            mcsq = small.tile([P, 1], f32, tag="mcsq")    # (S/H)^2
            V.scalar_tensor_tensor(out=mcsq[:], in0=sc1[:], scalar=sc1[:],
                                   in1=invh2[:], op0=OP.mult, op1=OP.mult)
            vc = small.tile([P, 1], f32, tag="vc")
            V.tensor_scalar(out=vc[:], in0=sqc1[:], scalar1=1.0 / H,
                            scalar2=EPS, op0=OP.mult, op1=OP.add)
            V.tensor_tensor(out=vc[:], in0=vc[:], in1=mcsq[:], op=OP.subtract)
            rc = _rsqrt(nc, small, magic1, vc, "c")
            bcn = small.tile([P, 1], f32, tag="bcn")      # -(S/H)*rc
            V.scalar_tensor_tensor(out=bcn[:], in0=sc1[:], scalar=rc[:],
                                   in1=neginvh[:], op0=OP.mult, op1=OP.mult)

            # ---- h1 = so * tanh(LN(c1)) --------------------------------
            th = gate.tile([P, H], f32, tag="th")
            if general:
                lnc = gate.tile([P, H], f32, tag="lnc")
                nc.scalar.activation(out=lnc[:], in_=c1t[:], func=AF.Identity,
                                     bias=bcn[:], scale=rc[:])
                nc.vector.tensor_mul(out=lnc[:], in0=lnc[:], in1=gbc_sb[:, 0, :])
                nc.vector.tensor_add(out=lnc[:], in0=lnc[:], in1=gbc_sb[:, 1, :])
                nc.scalar.activation(out=th[:], in_=lnc[:], func=AF.Tanh)
            else:
                nc.scalar.activation(out=th[:], in_=c1t[:], func=AF.Tanh,
                                     bias=bcn[:], scale=rc[:])
            h1t = io.tile([P, H], f32, tag="h1t")
            gp.tensor_tensor(out=h1t[:], in0=sfio[:, 2 * H:3 * H], in1=th[:],
                             op=OP.mult)
            nc.sync.dma_start(out=h1_r[t], in_=h1t[:])

    if not nc.is_finalized():
        nc.finalize()
    return nc


def _prep_core_inputs(x, h0, c0, w_ih, w_hh, bias_x, bias_h, kc_chunks,
                      general, g_ih, b_ih, g_hh, b_hh, g_c, b_c):
    """Build in_maps for the 8 cores (host-side shard + transpose + pad)."""
    KI = kc_chunks * P
    need_pad = KI != I
    if MM_DTYPE == "bfloat16":
        import ml_dtypes
        mm_np = ml_dtypes.bfloat16
    else:
        mm_np = np.float32

    def pad_feat(mT, bias):
        # mT: [I, BC] -> [KI, BC]; appended row of ones picks up the bias row
        out = np.zeros((KI, mT.shape[1]), dtype=mm_np)
        out[:I] = mT
        if need_pad:
            out[I] = 1.0
        return out

    def pad_w(w, bias):
        out = np.zeros((KI, N4H), dtype=mm_np)
        out[:I] = w
        if need_pad:
            out[I] = bias
        return out

    w_ih_p = pad_w(w_ih, bias_x)
    w_hh_p = pad_w(w_hh, bias_h)
    gb = np.stack([g_hh, b_hh, g_ih, b_ih]).astype(np.float32)
    gbc = np.stack([g_c, b_c]).astype(np.float32)

    in_maps = []
    for c in range(NCORES):
        sl = slice(c * BC, (c + 1) * BC)
        m = {
            "xT": pad_feat(np.ascontiguousarray(x[sl].T), bias_x),
            "hT": pad_feat(np.ascontiguousarray(h0[sl].T), bias_h),
            "c0s": np.ascontiguousarray(c0[sl]),
            "w_ih": w_ih_p,
            "w_hh": w_hh_p,
        }
        if general:
            m["gb"] = gb
            m["gbc"] = gbc
        in_maps.append(m)
    return in_maps


def kernel(x, h0, c0, w_ih, w_hh, bias_x, bias_h, g_ih, b_ih, g_hh, b_hh,
           g_c, b_c, _results_hook=None):
    x = np.asarray(x, dtype=np.float32)
    h0 = np.asarray(h0, dtype=np.float32)
    c0 = np.asarray(c0, dtype=np.float32)
    w_ih = np.asarray(w_ih, dtype=np.float32)
    w_hh = np.asarray(w_hh, dtype=np.float32)
    bias_x = np.asarray(bias_x, dtype=np.float32)
    bias_h = np.asarray(bias_h, dtype=np.float32)
    g_ih = np.asarray(g_ih, dtype=np.float32)
    b_ih = np.asarray(b_ih, dtype=np.float32)
    g_hh = np.asarray(g_hh, dtype=np.float32)
    b_hh = np.asarray(b_hh, dtype=np.float32)
    g_c = np.asarray(g_c, dtype=np.float32)
    b_c = np.asarray(b_c, dtype=np.float32)

    have_bias = bool(np.any(bias_x) or np.any(bias_h))
    kc_chunks = 5 if have_bias else 4
    general = not (
        np.all(g_ih == 1) and np.all(g_hh == 1) and np.all(g_c == 1)
        and not np.any(b_ih) and not np.any(b_hh) and not np.any(b_c)
    )

    key = (kc_chunks, general)
    if key not in _built:
        _built[key] = _build(kc_chunks, general)
    nc = _built[key]

    in_maps = _prep_core_inputs(x, h0, c0, w_ih, w_hh, bias_x, bias_h,
                                kc_chunks, general, g_ih, b_ih, g_hh, b_hh,
                                g_c, b_c)

    from concourse.bass_utils import run_bass_kernel_spmd

    res = run_bass_kernel_spmd(nc, in_maps, list(range(NCORES)))
    if _results_hook is not None:
        _results_hook(res)

    h1 = np.concatenate([res.results[c]["h1s"] for c in range(NCORES)], axis=0)
    c1 = np.concatenate([res.results[c]["c1s"] for c in range(NCORES)], axis=0)
    return h1, c1
